# revision 1
# baseline (speedup 1.0000x reference)
"""ECE loss kernel for Trainium2 (Bass/Tile), data-parallel over 8 NeuronCores.

Math (per sample row of logits[N, C]):
  conf = max softmax(x) = exp(max(x)) / sum(exp(x))
  acc  = (argmax(x) == label)
  bin1 = ceil(conf * 15)            # in {1..16}; ref bin index = bin1 - 1
  ece  = sum_b |conf_sum[b] - acc_sum[b]| / N

Device work per core (125k rows as [125 partitions x 1000 samples]):
  - m = rowmax(x)      (DVE reduce, 20 tiles of [125, 50, 100])
  - E = exp(x)         (ACT)
  - s = rowsum(E)      (DVE reduce)
  - conf = exp(m) * (1/s);  acc = (g == m)  where g = x[i, label_i] is
    gathered on the host (tiny input, 1% of data) to avoid a device gather.
  - binning via cumulative masked sums (one tensor_scalar+accum per bin):
      wt[b] = sum min(conf, C_b)         -> conf-weighted cumulative (algebra below)
      nn[b] = sum (conf <= C_b)
      av[b] = sum (v <= 2 + C_b), v = conf + 2*acc   -> acc cumulative
  Host recovers per-bin sums:
      S_b = wt_b - C_b * (n - nn_b)  (= sum of conf over bin1 <= b+1)
      A_b = av_b - #acc0
      conf_sum[k] = S_k - S_{k-1};  acc_sum[k] = A_k - A_{k-1}
C_b is the exact f32 boundary: the largest f32 y with f32(15*y) <= b+1, so
binning matches the reference's ceil(conf*15) bit-for-bit.
"""

import os

import numpy as np

import concourse.bass as bass
import concourse.mybir as mybir
import concourse.tile as tile
from concourse.bass_utils import run_bass_kernel_spmd

F32 = mybir.dt.float32
ALU = mybir.AluOpType
AX = mybir.AxisListType
ACTF = mybir.ActivationFunctionType

N = 1_000_000
C = 100
NCORES = 8
ROWS = N // NCORES          # 125000 rows per core
P = 125                     # SBUF partitions used
SPP = ROWS // P             # 1000 samples per partition
TILE_K = 50                 # samples per partition per tile
NTILES = SPP // TILE_K      # tiles per core
NBINS = 16                  # 15 real bins + always-empty tail bin

LAST_RESULTS = None         # stashed BassKernelResults for test harness


def _bin_thresholds():
    """C_b = largest f32 y such that f32(15*y) <= b+1, for b = 0..14."""
    thr = []
    for b in range(15):
        tgt = np.float32(b + 1)

        def f(v):
            return np.float32(np.float32(15.0) * v)

        y = np.float32((b + 1) / 15.0)
        if f(y) <= tgt:
            while True:
                y2 = np.nextafter(y, np.float32(np.inf))
                if f(y2) <= tgt:
                    y = y2
                else:
                    break
        else:
            while f(y) > tgt:
                y = np.nextafter(y, np.float32(-np.inf))
        thr.append(np.float32(y))
    thr.append(np.float32(1e9))  # catch-all last segment
    return thr


THR = _bin_thresholds()


def _build():
    nc = bass.Bass(trn_type="TRN2")
    x = nc.dram_tensor("x", [P, SPP * C], F32, kind="ExternalInput")
    g = nc.dram_tensor("g", [P, SPP], F32, kind="ExternalInput")
    wt = nc.dram_tensor("wt", [P, NBINS], F32, kind="ExternalOutput")
    nn_ = nc.dram_tensor("nn", [P, NBINS], F32, kind="ExternalOutput")
    av = nc.dram_tensor("av", [P, NBINS], F32, kind="ExternalOutput")

    X = x[:, :].rearrange("p (k c) -> p k c", c=C)  # [125, 1000, 100]

    # small leading tiles so the first transfers land quickly and the
    # compute pipeline starts sooner
    sizes = [12, 13, 25] + [TILE_K] * 19
    assert sum(sizes) == SPP
    BUFS = 5

    with tile.TileContext(nc) as tc:
        with (
            tc.tile_pool(name="xin", bufs=BUFS) as xin,
            tc.tile_pool(name="hpool", bufs=BUFS) as hpool,
            tc.tile_pool(name="persist", bufs=1) as persist,
            tc.tile_pool(name="scr", bufs=1) as scr,
        ):
            em_all = persist.tile([P, SPP], F32)
            s_all = persist.tile([P, SPP], F32)
            g_sb = persist.tile([P, SPP], F32)
            nc.gpsimd.dma_start(out=g_sb[:, :], in_=g[:, :])

            dma_engines = [nc.sync, nc.scalar, nc.gpsimd]
            off = 0
            for t, k in enumerate(sizes):
                sl = slice(off, off + k)
                off += k
                xt = xin.tile([P, TILE_K, C], F32, tag="xt")
                # spread loads across engine DGEs so transfers overlap
                dma_engines[t % len(dma_engines)].dma_start(
                    out=xt[:, :k, :], in_=X[:, sl, :]
                )
                # exp in place: per xt slot the chain is DMA -> ACT -> DVE
                # (instruction encodings only have 2 sync-command slots, so
                # the dependency structure must be a single chain per slot).
                # Row-max is taken over E = exp(x): exp is monotone and the
                # accuracy compare uses the same spline output.
                nc.scalar.activation(xt[:, :k, :], xt[:, :k, :], ACTF.Exp)
                nc.vector.reduce_sum(out=s_all[:, sl], in_=xt[:, :k, :], axis=AX.X)
                nc.vector.reduce_max(out=em_all[:, sl], in_=xt[:, :k, :], axis=AX.X)

            # tail: conf = exp(m) / s, acc = (exp(g) == exp(m)); all the
            # [P, SPP] temps are reused in place to stay inside SBUF
            nc.vector.reciprocal(s_all[:, :], s_all[:, :])
            nc.scalar.activation(g_sb[:, :], g_sb[:, :], ACTF.Exp)
            acc = g_sb
            nc.vector.tensor_tensor(
                acc[:, :], g_sb[:, :], em_all[:, :], op=ALU.is_equal
            )
            conf = em_all
            nc.vector.tensor_mul(conf[:, :], em_all[:, :], s_all[:, :])

            wt_sb = persist.tile([P, NBINS], F32)
            nn_sb = persist.tile([P, NBINS], F32)
            av_sb = persist.tile([P, NBINS], F32)
            for b in range(NBINS):
                cb = float(THR[b])
                s1 = scr.tile([P, SPP], F32, tag="s1")
                nc.vector.tensor_scalar(
                    s1[:, :], conf[:, :], cb, None,
                    op0=ALU.min, op1=ALU.add, accum_out=wt_sb[:, b : b + 1],
                )
                s2 = scr.tile([P, SPP], F32, tag="s2")
                nc.vector.tensor_scalar(
                    s2[:, :], conf[:, :], cb, None,
                    op0=ALU.is_le, op1=ALU.add, accum_out=nn_sb[:, b : b + 1],
                )
                s3 = scr.tile([P, SPP], F32, tag="s3")
                nc.vector.scalar_tensor_tensor(
                    s3[:, :], conf[:, :], cb, acc[:, :],
                    op0=ALU.is_le, op1=ALU.mult, accum_out=av_sb[:, b : b + 1],
                )

            nc.sync.dma_start(out=wt[:, :], in_=wt_sb[:, :])
            nc.sync.dma_start(out=nn_[:, :], in_=nn_sb[:, :])
            nc.sync.dma_start(out=av[:, :], in_=av_sb[:, :])

    # Instruction encodings only have 2 sync-command slots (completion
    # update takes one), so every instruction must carry <= 1 wait.  The
    # per-slot dependency chain is DMA -> ACT exp -> GpSimd halve -> DVE
    # rsum -> DVE rmax; each link's wait transitively implies every earlier
    # link, so redundant waits are dropped here.
    import re as _re

    def _tick_sem(name):
        # monotone per-engine tick / DMA-queue sems; barrier event sems are
        # decremented and reused so they must never be touched
        return bool(_re.match(r"^(Activation|DVE|PE|Pool|SP|DMAHW\d+|DMASW\d+)_\d+$", name))

    seen_waits = {}
    for bb in nc.m.functions[0].blocks:
        for ins in bb.instructions:
            si = ins.sync_info
            if si is None:
                continue
            tname = type(ins).__name__
            if tname == "InstEventSemaphore":
                continue
            eng = str(ins.engine).split(".")[-1]
            kept = list(si.on_wait)
            if tname == "InstDMACopy" and "@xt" in ins.concise():
                # keep only the DVE wait (rmax of the reused slot; it ran
                # after rsum -> after the gpsimd halve -> after exp)
                dve = [w for w in kept if w.ant_name.startswith("DVE")]
                if dve:
                    kept = dve
            elif tname == "InstDMACopy":
                non_q = [
                    w for w in kept
                    if not w.ant_name.startswith(("DMAHW", "DMASW"))
                ]
                if non_q:
                    kept = non_q
            elif tname == "InstActivation" and "@xt" in ins.concise():
                # in-place exp: WAR on the slot's old readers is enforced by
                # this slot's input DMA already; keep only the DMA wait
                q = [w for w in kept if w.ant_name.startswith(("DMAHW", "DMASW"))]
                if q:
                    kept = q
            elif tname == "InstTensorTensor" and eng == "Pool":
                # gpsimd halve: its ACT wait (exp of this slot) implies the
                # slot-reuse DVE wait via exp's own input-DMA chain
                act = [w for w in kept if w.ant_name.startswith("Activation")]
                if act:
                    kept = act
            if tname not in ("InstDMACopy", "InstDrain") and len(kept) > 1:
                # same-engine waits are redundant (program order); drop them
                # only when the instruction exceeds its single wait slot
                kept = [w for w in kept if not w.ant_name.startswith(f"{eng}_")]
            # drop waits already covered by an earlier wait on this engine
            kept2 = []
            for w in kept:
                if not _tick_sem(w.ant_name):
                    kept2.append(w)
                elif seen_waits.get((eng, w.ant_name), -1) < w.wait_value:
                    kept2.append(w)
            kept = kept2
            for w in kept:
                if not _tick_sem(w.ant_name):
                    continue
                key = (eng, w.ant_name)
                seen_waits[key] = max(seen_waits.get(key, -1), w.wait_value)
            if len(kept) != len(si.on_wait):
                si.on_wait = kept
                ins.sync_info = si

    import bass_rust as _br

    # Instructions may carry at most 2 sync commands (waits + updates).  The
    # Tile kernel-tail drain waits on every DMA queue sem at once — split its
    # wait list across a chain of preceding same-engine drains.

    for bb in nc.m.functions[0].blocks:
        while True:
            insns = list(bb.instructions)
            target = None
            for idx, ins in enumerate(insns):
                si = ins.sync_info
                if si is None:
                    continue
                if len(si.on_wait) > 1:
                    target = (idx, ins)
                    break
            if target is None:
                break
            idx, ins = target
            # drains fit a single sync command; other engine instructions fit
            # one wait + their completion update
            si = ins.sync_info
            waits = list(si.on_wait)
            if type(ins).__name__ == "InstDrain":
                room = max(0, 1 - len(si.on_update))
            else:
                room = 1
            keep, extra = waits[len(waits) - room :], waits[: len(waits) - room]
            pos = idx
            for i, w in enumerate(extra):
                nd = mybir.InstDrain(
                    name=f"{ins.name}-presync{i}", ins=[], outs=[],
                    bass_is_fusable=False,
                )
                nd.engine = ins.engine
                nd.sync_info = _br.SyncInfo(on_wait=[w], on_update=[])
                nc.register_instruction(nd, overwrite=True)
                bb.instructions.insert(pos, nd)
                pos += 1
            si.on_wait = keep
            ins.sync_info = si
    return nc


_NC_CACHE = {}


def _get_nc():
    if "nc" not in _NC_CACHE:
        _NC_CACHE["nc"] = _build()
    return _NC_CACHE["nc"]


def kernel(logits, labels):
    global LAST_RESULTS
    logits = np.ascontiguousarray(np.asarray(logits), dtype=np.float32)
    labels_i = np.asarray(labels).astype(np.int64)
    assert logits.shape == (N, C), logits.shape

    # host-side gather of the label logit (1% of input bytes; the heavy
    # softmax/max/binning all happen on device)
    gvals = logits[np.arange(N), labels_i].astype(np.float32)

    in_maps = []
    for c in range(NCORES):
        sl = slice(c * ROWS, (c + 1) * ROWS)
        in_maps.append(
            {
                "x": logits[sl].reshape(P, SPP * C),
                "g": gvals[sl].reshape(P, SPP),
            }
        )

    trace = bool(int(os.environ.get("ECE_TRACE", "0")))
    res = run_bass_kernel_spmd(
        _get_nc(), in_maps, core_ids=list(range(NCORES)), trace=trace
    )
    LAST_RESULTS = res

    wt = np.zeros(NBINS, np.float64)
    nn_ = np.zeros(NBINS, np.float64)
    av = np.zeros(NBINS, np.float64)
    for out in res.results:
        wt += out["wt"].astype(np.float64).sum(axis=0)
        nn_ += out["nn"].astype(np.float64).sum(axis=0)
        av += out["av"].astype(np.float64).sum(axis=0)

    thr64 = np.array([np.float64(t) for t in THR])
    S = wt - thr64 * (N - nn_)
    S[15] = wt[15]
    conf_sum = np.diff(S, prepend=0.0)
    acc_sum = np.diff(av, prepend=0.0)
    ece = np.abs(conf_sum - acc_sum).sum() / N
    return np.array([ece], dtype=np.float32)

